# revision 4
# baseline (speedup 1.0000x reference)
"""Char-level BiLSTM embedder on 8 NeuronCores (Trainium2, Bass/Tile).

Computation: x[B=32,T=128,L=16] char ids -> embed[E=512] -> fwd+bwd LSTM(H=256)
over the L=16 chars of each of the N=B*T=4096 independent words -> final hidden
states concatenated -> y[B,T,2H=512].

Strategy:
  - Data parallel over N: 512 words per core.
  - Embedding lookup + input projection + bias fused on HOST into a single
    [V=128, 4H] LUT per direction:  fused[v,:] = embed[v] @ w_ih.T + b.
    On device the per-step input contribution is a K=128 matmul with a
    one-hot rhs (built on host), accumulated into the same PSUM group as
    the recurrent h matmuls.
  - Everything device-side is feature-major: gates/c/h live as
    [feature-chunk on partitions, words on free dim], so h feeds the next
    step's matmul rhs directly and no transposes are ever needed.
  - Gate order permuted to (i,f,o,g) so activations batch into 3 big ACT ops.
  - fwd and bwd directions interleave per step to hide recurrence latency.
  - Host does the final [2H,n] -> [n,2H] transpose and core concat.
"""

import sys

sys.path.insert(0, "/opt/trn_rl_repo")

import numpy as np
import concourse.bass as bass
import concourse.bacc as bacc
import concourse.mybir as mybir
import concourse.tile as tile
from concourse.bass_utils import run_bass_kernel_spmd

# problem constants (hardcoded per harness contract)
B, T, L = 32, 128, 16
VOCAB, E, H = 128, 512, 256
G4 = 4 * H  # 1024
N_CORES = 8
NW = (B * T) // N_CORES  # 512 words per core

F32 = mybir.dt.float32
DT = mybir.dt.float32  # compute dtype for matmul operands / activations

AFT = mybir.ActivationFunctionType


def build_nc():
    nc = bacc.Bacc()

    oh_d = nc.dram_tensor("oh", [L, VOCAB, NW], DT, kind="ExternalInput")
    fused_d = {
        d: nc.dram_tensor(f"fused_{d}", [VOCAB, G4], DT, kind="ExternalInput")
        for d in "fb"
    }
    whh_d = {
        d: nc.dram_tensor(f"whh_{d}", [2, 128, G4], DT, kind="ExternalInput")
        for d in "fb"
    }
    hout_d = nc.dram_tensor("hout", [4, 128, NW], F32, kind="ExternalOutput")

    with tile.TileContext(nc) as tc:
        with (
            tc.tile_pool(name="const", bufs=1) as cpool,
            tc.tile_pool(name="work", bufs=2) as wpool,
            tc.tile_pool(name="state", bufs=2) as spool,
            tc.tile_pool(name="psum", bufs=2, space=bass.MemorySpace.PSUM) as ppool,
        ):
            # --- load constants -------------------------------------------
            fused = {}
            whh = {}
            for d in "fb":
                fused[d] = cpool.tile([128, G4], DT, name=f"fused_{d}_sb", tag=f"fused_{d}")
                nc.sync.dma_start(fused[d][:], fused_d[d][:])
                whh[d] = []
                for k in range(2):
                    w = cpool.tile([128, G4], DT, name=f"whh_{d}{k}_sb", tag=f"whh_{d}{k}")
                    nc.sync.dma_start(w[:], whh_d[d][k])
                    whh[d].append(w)

            # one tile per char position, each with its own DMA (keeps the
            # per-matmul sync-wait count low), loaded in the order the two
            # directions will consume them
            load_order = []
            for t in range(L):
                for tc_ in (t, L - 1 - t):
                    if tc_ not in load_order:
                        load_order.append(tc_)
            oh_tiles = [None] * L
            for t in load_order:
                ot = cpool.tile([128, NW], DT, name=f"oh_{t}", tag=f"oh_{t}")
                nc.sync.dma_start(ot[:], oh_d[t])
                oh_tiles[t] = ot

            out_sb = cpool.tile([128, 4 * NW], F32, name="out_sb", tag="out_sb")

            c_cur = {"f": None, "b": None}
            h_cur = {"f": None, "b": None}

            # --- recurrent steps ------------------------------------------
            for t in range(L):
                for d in "fb":
                    tchar = t if d == "f" else L - 1 - t
                    rhs_oh = oh_tiles[tchar][:]
                    h_prev = h_cur[d]
                    c_prev = c_cur[d]

                    # gate chunks 0..3 = i0,i1,f0,f1 ; 4..7 = o0,o1,g0,g1
                    psum_a = ppool.tile([128, 4 * NW], F32, name="psum_a", tag="ps")
                    psum_b = ppool.tile([128, 4 * NW], F32, name="psum_b", tag="ps")
                    for half, ps in ((0, psum_a), (1, psum_b)):
                        for jj in range(4):
                            gc = half * 4 + jj  # global gate chunk 0..7
                            sl = ps[:, jj * NW : (jj + 1) * NW]
                            lhs_f = fused[d][:, gc * 128 : (gc + 1) * 128]
                            if h_prev is None:
                                nc.tensor.matmul(sl, lhs_f, rhs_oh, start=True, stop=True)
                            else:
                                nc.tensor.matmul(sl, lhs_f, rhs_oh, start=True, stop=False)
                                for k in range(2):
                                    lhs_h = whh[d][k][:, gc * 128 : (gc + 1) * 128]
                                    rhs_h = h_prev[:, k * NW : (k + 1) * NW]
                                    nc.tensor.matmul(
                                        sl, lhs_h, rhs_h, start=False, stop=(k == 1)
                                    )

                    # activations
                    sig_if = wpool.tile([128, 4 * NW], DT, name="sig_if", tag="sig_if")
                    nc.scalar.activation(sig_if[:], psum_a[:], AFT.Sigmoid)
                    sig_o = wpool.tile([128, 2 * NW], DT, name="sig_o", tag="sig_o")
                    nc.scalar.activation(sig_o[:], psum_b[:, 0 : 2 * NW], AFT.Sigmoid)
                    tanh_g = wpool.tile([128, 2 * NW], DT, name="tanh_g", tag="tanh_g")
                    nc.scalar.activation(tanh_g[:], psum_b[:, 2 * NW : 4 * NW], AFT.Tanh)

                    # cell update: c = sig(f) * c + sig(i) * tanh(g)
                    c_new = spool.tile([128, 2 * NW], F32, name=f"c_{d}", tag=f"c_{d}")
                    if c_prev is None:
                        nc.vector.tensor_mul(c_new[:], sig_if[:, 0 : 2 * NW], tanh_g[:])
                    else:
                        m2 = wpool.tile([128, 2 * NW], DT, name="m2", tag="m2")
                        nc.vector.tensor_mul(m2[:], sig_if[:, 0 : 2 * NW], tanh_g[:])
                        m1 = wpool.tile([128, 2 * NW], F32, name="m1", tag="m1")
                        nc.vector.tensor_mul(m1[:], sig_if[:, 2 * NW : 4 * NW], c_prev[:])
                        nc.vector.tensor_add(c_new[:], m1[:], m2[:])
                    c_cur[d] = c_new

                    tanh_c = wpool.tile([128, 2 * NW], DT, name="tanh_c", tag="tanh_c")
                    nc.scalar.activation(tanh_c[:], c_new[:], AFT.Tanh)

                    if t == L - 1:
                        off = 0 if d == "f" else 2 * NW
                        nc.vector.tensor_mul(
                            out_sb[:, off : off + 2 * NW], sig_o[:], tanh_c[:]
                        )
                    else:
                        h_new = spool.tile([128, 2 * NW], DT, name=f"h_{d}", tag=f"h_{d}")
                        nc.vector.tensor_mul(h_new[:], sig_o[:], tanh_c[:])
                        h_cur[d] = h_new

            for q in range(4):
                nc.sync.dma_start(hout_d[q], out_sb[:, q * NW : (q + 1) * NW])

    nc.compile()
    return nc


_NC_CACHE = None


def _get_nc():
    global _NC_CACHE
    if _NC_CACHE is None:
        _NC_CACHE = build_nc()
    return _NC_CACHE


# gate permutation: torch order (i,f,g,o) -> device order (i,f,o,g)
_PERM = np.concatenate([np.arange(0, 512), np.arange(768, 1024), np.arange(512, 768)])


def _np_dt(dt):
    return mybir.dt.np(dt)


def prepare_in_maps(x, embed_table, w_ih_f, w_hh_f, b_ih_f, b_hh_f,
                    w_ih_b, w_hh_b, b_ih_b, b_hh_b):
    cdt = _np_dt(DT)
    ids = np.asarray(x).reshape(B * T, L).astype(np.int64)

    shared = {}
    for d, w_ih, w_hh, b_ih, b_hh in (
        ("f", w_ih_f, w_hh_f, b_ih_f, b_hh_f),
        ("b", w_ih_b, w_hh_b, b_ih_b, b_hh_b),
    ):
        w_ih = np.asarray(w_ih, np.float32)[_PERM]
        w_hh = np.asarray(w_hh, np.float32)[_PERM]
        b = (np.asarray(b_ih, np.float32) + np.asarray(b_hh, np.float32))[_PERM]
        fused = np.asarray(embed_table, np.float32) @ w_ih.T + b[None, :]
        shared[f"fused_{d}"] = np.ascontiguousarray(fused.astype(cdt))
        shared[f"whh_{d}"] = np.ascontiguousarray(
            w_hh.T.reshape(2, 128, G4).astype(cdt)
        )

    vrange = np.arange(VOCAB)
    in_maps = []
    for c in range(N_CORES):
        ids_c = ids[c * NW : (c + 1) * NW]  # [NW, L]
        oh = (ids_c.T[:, None, :] == vrange[None, :, None]).astype(cdt)  # [L,V,NW]
        m = dict(shared)
        m["oh"] = np.ascontiguousarray(oh)
        in_maps.append(m)
    return in_maps


def assemble_output(results):
    ys = []
    for c in range(N_CORES):
        hout = results[c]["hout"].astype(np.float32)  # [4,128,NW]
        hf = hout[0:2].reshape(2 * 128, NW)  # [H, NW]
        hb = hout[2:4].reshape(2 * 128, NW)
        ys.append(np.concatenate([hf.T, hb.T], axis=1))  # [NW, 2H]
    y = np.concatenate(ys, axis=0)  # [B*T, 2H]
    return y.reshape(B, T, 2 * H)


def run(in_maps, trace=False):
    nc = _get_nc()
    res = run_bass_kernel_spmd(nc, in_maps, core_ids=list(range(N_CORES)), trace=trace)
    return res


def kernel(**inputs) -> np.ndarray:
    in_maps = prepare_in_maps(**inputs)
    res = run(in_maps, trace=False)
    return assemble_output(res.results)


# revision 5
# speedup vs baseline: 2.8309x; 2.8309x over previous
"""Char-level BiLSTM embedder on 8 NeuronCores (Trainium2, Bass/Tile).

Computation: x[B=32,T=128,L=16] char ids -> embed[E=512] -> fwd+bwd LSTM(H=256)
over the L=16 chars of each of the N=B*T=4096 independent words -> final hidden
states concatenated -> y[B,T,2H=512].

Strategy:
  - Data parallel over N: 512 words per core.
  - Embedding lookup + input projection + bias fused on HOST into a single
    [V=128, 4H] LUT per direction:  fused[v,:] = embed[v] @ w_ih.T + b.
    On device the per-step input contribution is a K=128 matmul with a
    one-hot rhs (built on host), accumulated into the same PSUM group as
    the recurrent h matmuls.
  - Everything device-side is feature-major: gates/c/h live as
    [feature-chunk on partitions, words on free dim], so h feeds the next
    step's matmul rhs directly and no transposes are ever needed.
  - Gate order permuted to (i,f,o,g) so activations batch into 3 big ACT ops.
  - fwd and bwd directions interleave per step to hide recurrence latency.
  - Host does the final [2H,n] -> [n,2H] transpose and core concat.
"""

import sys

sys.path.insert(0, "/opt/trn_rl_repo")

import numpy as np
import concourse.bass as bass
import concourse.bacc as bacc
import concourse.mybir as mybir
import concourse.tile as tile
from concourse.bass_utils import run_bass_kernel_spmd

# problem constants (hardcoded per harness contract)
B, T, L = 32, 128, 16
VOCAB, E, H = 128, 512, 256
G4 = 4 * H  # 1024
N_CORES = 8
NW = (B * T) // N_CORES  # 512 words per core

F32 = mybir.dt.float32
# compute dtype for matmul operands / gate activations. bf16 halves PE time
# (fp32 matmuls decompose into 2 passes) and enables DVE 2x modes; the cell
# state c and all PSUM accumulation stay fp32.
DT = mybir.dt.bfloat16

AFT = mybir.ActivationFunctionType


def build_nc():
    nc = bacc.Bacc()

    oh_d = nc.dram_tensor("oh", [L, VOCAB, NW], DT, kind="ExternalInput")
    fused_d = {
        d: nc.dram_tensor(f"fused_{d}", [VOCAB, G4], DT, kind="ExternalInput")
        for d in "fb"
    }
    whh_d = {
        d: nc.dram_tensor(f"whh_{d}", [2, 128, G4], DT, kind="ExternalInput")
        for d in "fb"
    }
    hout_d = nc.dram_tensor("hout", [4, 128, NW], F32, kind="ExternalOutput")

    with tile.TileContext(nc) as tc:
        with (
            tc.tile_pool(name="const", bufs=1) as cpool,
            tc.tile_pool(name="work", bufs=2) as wpool,
            tc.tile_pool(name="state", bufs=2) as spool,
            tc.tile_pool(name="psum", bufs=2, space=bass.MemorySpace.PSUM) as ppool,
        ):
            # --- load constants -------------------------------------------
            fused = {}
            whh = {}
            for d in "fb":
                fused[d] = cpool.tile([128, G4], DT, name=f"fused_{d}_sb", tag=f"fused_{d}")
                nc.sync.dma_start(fused[d][:], fused_d[d][:])
                whh[d] = []
                for k in range(2):
                    w = cpool.tile([128, G4], DT, name=f"whh_{d}{k}_sb", tag=f"whh_{d}{k}")
                    nc.sync.dma_start(w[:], whh_d[d][k])
                    whh[d].append(w)

            # one tile per char position, each with its own DMA (keeps the
            # per-matmul sync-wait count low), loaded in the order the two
            # directions will consume them
            load_order = []
            for t in range(L):
                for tc_ in (t, L - 1 - t):
                    if tc_ not in load_order:
                        load_order.append(tc_)
            oh_tiles = [None] * L
            for t in load_order:
                ot = cpool.tile([128, NW], DT, name=f"oh_{t}", tag=f"oh_{t}")
                nc.sync.dma_start(ot[:], oh_d[t])
                oh_tiles[t] = ot

            out_sb = cpool.tile([128, 4 * NW], F32, name="out_sb", tag="out_sb")

            c_cur = {"f": None, "b": None}
            h_cur = {"f": None, "b": None}

            # --- recurrent steps ------------------------------------------
            for t in range(L):
                for d in "fb":
                    tchar = t if d == "f" else L - 1 - t
                    rhs_oh = oh_tiles[tchar][:]
                    h_prev = h_cur[d]
                    c_prev = c_cur[d]

                    # gate chunks 0..3 = i0,i1,f0,f1 ; 4..7 = o0,o1,g0,g1
                    psum_a = ppool.tile([128, 4 * NW], F32, name="psum_a", tag="ps")
                    psum_b = ppool.tile([128, 4 * NW], F32, name="psum_b", tag="ps")
                    for half, ps in ((0, psum_a), (1, psum_b)):
                        for jj in range(4):
                            gc = half * 4 + jj  # global gate chunk 0..7
                            sl = ps[:, jj * NW : (jj + 1) * NW]
                            lhs_f = fused[d][:, gc * 128 : (gc + 1) * 128]
                            if h_prev is None:
                                nc.tensor.matmul(sl, lhs_f, rhs_oh, start=True, stop=True)
                            else:
                                nc.tensor.matmul(sl, lhs_f, rhs_oh, start=True, stop=False)
                                for k in range(2):
                                    lhs_h = whh[d][k][:, gc * 128 : (gc + 1) * 128]
                                    rhs_h = h_prev[:, k * NW : (k + 1) * NW]
                                    nc.tensor.matmul(
                                        sl, lhs_h, rhs_h, start=False, stop=(k == 1)
                                    )

                    # activations
                    sig_if = wpool.tile([128, 4 * NW], DT, name="sig_if", tag="sig_if")
                    nc.scalar.activation(sig_if[:], psum_a[:], AFT.Sigmoid)
                    sig_o = wpool.tile([128, 2 * NW], DT, name="sig_o", tag="sig_o")
                    nc.scalar.activation(sig_o[:], psum_b[:, 0 : 2 * NW], AFT.Sigmoid)
                    tanh_g = wpool.tile([128, 2 * NW], DT, name="tanh_g", tag="tanh_g")
                    nc.scalar.activation(tanh_g[:], psum_b[:, 2 * NW : 4 * NW], AFT.Tanh)

                    # cell update: c = sig(f) * c + sig(i) * tanh(g)
                    c_new = spool.tile([128, 2 * NW], F32, name=f"c_{d}", tag=f"c_{d}")
                    if c_prev is None:
                        nc.vector.tensor_mul(c_new[:], sig_if[:, 0 : 2 * NW], tanh_g[:])
                    else:
                        m2 = wpool.tile([128, 2 * NW], DT, name="m2", tag="m2")
                        nc.vector.tensor_mul(m2[:], sig_if[:, 0 : 2 * NW], tanh_g[:])
                        m1 = wpool.tile([128, 2 * NW], F32, name="m1", tag="m1")
                        nc.vector.tensor_mul(m1[:], sig_if[:, 2 * NW : 4 * NW], c_prev[:])
                        nc.vector.tensor_add(c_new[:], m1[:], m2[:])
                    c_cur[d] = c_new

                    tanh_c = wpool.tile([128, 2 * NW], DT, name="tanh_c", tag="tanh_c")
                    nc.scalar.activation(tanh_c[:], c_new[:], AFT.Tanh)

                    if t == L - 1:
                        off = 0 if d == "f" else 2 * NW
                        nc.vector.tensor_mul(
                            out_sb[:, off : off + 2 * NW], sig_o[:], tanh_c[:]
                        )
                    else:
                        h_new = spool.tile([128, 2 * NW], DT, name=f"h_{d}", tag=f"h_{d}")
                        nc.vector.tensor_mul(h_new[:], sig_o[:], tanh_c[:])
                        h_cur[d] = h_new

            for q in range(4):
                nc.sync.dma_start(hout_d[q], out_sb[:, q * NW : (q + 1) * NW])

    nc.compile()
    return nc


_NC_CACHE = None


def _get_nc():
    global _NC_CACHE
    if _NC_CACHE is None:
        _NC_CACHE = build_nc()
    return _NC_CACHE


# gate permutation: torch order (i,f,g,o) -> device order (i,f,o,g)
_PERM = np.concatenate([np.arange(0, 512), np.arange(768, 1024), np.arange(512, 768)])


def _np_dt(dt):
    return mybir.dt.np(dt)


def prepare_in_maps(x, embed_table, w_ih_f, w_hh_f, b_ih_f, b_hh_f,
                    w_ih_b, w_hh_b, b_ih_b, b_hh_b):
    cdt = _np_dt(DT)
    ids = np.asarray(x).reshape(B * T, L).astype(np.int64)

    shared = {}
    for d, w_ih, w_hh, b_ih, b_hh in (
        ("f", w_ih_f, w_hh_f, b_ih_f, b_hh_f),
        ("b", w_ih_b, w_hh_b, b_ih_b, b_hh_b),
    ):
        w_ih = np.asarray(w_ih, np.float32)[_PERM]
        w_hh = np.asarray(w_hh, np.float32)[_PERM]
        b = (np.asarray(b_ih, np.float32) + np.asarray(b_hh, np.float32))[_PERM]
        fused = np.asarray(embed_table, np.float32) @ w_ih.T + b[None, :]
        shared[f"fused_{d}"] = np.ascontiguousarray(fused.astype(cdt))
        shared[f"whh_{d}"] = np.ascontiguousarray(
            w_hh.T.reshape(2, 128, G4).astype(cdt)
        )

    vrange = np.arange(VOCAB)
    in_maps = []
    for c in range(N_CORES):
        ids_c = ids[c * NW : (c + 1) * NW]  # [NW, L]
        oh = (ids_c.T[:, None, :] == vrange[None, :, None]).astype(cdt)  # [L,V,NW]
        m = dict(shared)
        m["oh"] = np.ascontiguousarray(oh)
        in_maps.append(m)
    return in_maps


def assemble_output(results):
    ys = []
    for c in range(N_CORES):
        hout = results[c]["hout"].astype(np.float32)  # [4,128,NW]
        hf = hout[0:2].reshape(2 * 128, NW)  # [H, NW]
        hb = hout[2:4].reshape(2 * 128, NW)
        ys.append(np.concatenate([hf.T, hb.T], axis=1))  # [NW, 2H]
    y = np.concatenate(ys, axis=0)  # [B*T, 2H]
    return y.reshape(B, T, 2 * H)


def run(in_maps, trace=False):
    nc = _get_nc()
    res = run_bass_kernel_spmd(nc, in_maps, core_ids=list(range(N_CORES)), trace=trace)
    return res


def kernel(**inputs) -> np.ndarray:
    in_maps = prepare_in_maps(**inputs)
    res = run(in_maps, trace=False)
    return assemble_output(res.results)
